# revision 2
# baseline (speedup 1.0000x reference)
"""Bass/Tile kernel for batched cross-attention on 8 TRN2 NeuronCores.

Problem (per reference):
    encoder_output: [S=2048, B=8, H=1024] f32
    decoder_hidden: [T=2048, B=8, H=1024] f32
    energies[b,t,s] = dec[t,b,:] . enc[s,b,:]
    weights = softmax(energies, axis=s)           -> returned as [T, B, S]
    context[t,b,:] = sum_s weights[b,t,s]*enc[s,b,:]  -> [T, B, H]

Sharding: batch b -> core b (pure data parallel, no collectives).

Per-core pipeline (T tiles of 128 rows):
    MM1 (TensorE, bf16):  E = dec_b @ enc_b^T     [128, S] fp32 in PSUM
    VectorE:              rowmax -> negmax
    ScalarE:              w = exp(E - max) (bf16) + accumulated rowsum
    DMA xbar:             w^T tiles for MM2
    TensorE (bf16):       C = w @ enc_b           [128, H] fp32 in PSUM
    ScalarE:              weights_out = w * (1/sum)  (f32), ctx_out = C * (1/sum)
The t-tile loop is software-pipelined (MM1 of tile i+1 is issued before
MM2 of tile i-1) so the PE stream never waits on the softmax/transpose.
"""

import os
import sys

for _p in ("/opt/trn_rl_repo", "/root/.axon_site/_ro/trn_rl_repo"):
    if os.path.isdir(_p) and _p not in sys.path:
        sys.path.insert(0, _p)

from contextlib import ExitStack

import ml_dtypes
import numpy as np

import concourse.bass as bass
import concourse.tile as tile
from concourse import bacc, mybir
from concourse.bass_utils import run_bass_kernel_spmd

S, T, B, H = 2048, 2048, 8, 1024
P = 128  # SBUF partitions
NBANK = 512  # fp32 PSUM bank free size / max moving free dim

BF16 = mybir.dt.bfloat16
F32 = mybir.dt.float32
AX_X = mybir.AxisListType.X
OP_MAX = mybir.AluOpType.max
OP_ADD = mybir.AluOpType.add
ACT_EXP = mybir.ActivationFunctionType.Exp
ACT_COPY = mybir.ActivationFunctionType.Copy


def build_kernel(t_dim=T, s_dim=S, h_dim=H):
    """Build and compile the per-core Bass module (same program on all cores)."""
    assert t_dim % P == 0 and s_dim % NBANK == 0 and h_dim % NBANK == 0

    kh = h_dim // P  # contraction chunks for MM1
    ks = s_dim // P  # contraction chunks for MM2
    nt = t_dim // P  # t tiles
    ns = s_dim // NBANK  # PSUM banks for energies
    nh = h_dim // NBANK  # PSUM-bank halves for context

    nc = bacc.Bacc("TRN2", target_bir_lowering=False, debug=False, num_devices=8)

    encT_d = nc.dram_tensor("encT", [h_dim, s_dim], BF16, kind="ExternalInput").ap()
    decT_d = nc.dram_tensor("decT", [h_dim, t_dim], BF16, kind="ExternalInput").ap()
    enc_d = nc.dram_tensor("enc", [s_dim, h_dim], BF16, kind="ExternalInput").ap()
    wout_d = nc.dram_tensor("weights", [t_dim, s_dim], F32, kind="ExternalOutput").ap()
    cout_d = nc.dram_tensor("context", [t_dim, h_dim], F32, kind="ExternalOutput").ap()

    with tile.TileContext(nc) as tc, ExitStack() as ctx:
        resident = ctx.enter_context(tc.tile_pool(name="resident", bufs=1))
        epool = ctx.enter_context(tc.tile_pool(name="epool", bufs=1, space="PSUM"))
        cpool = ctx.enter_context(tc.tile_pool(name="cpool", bufs=1, space="PSUM"))
        wbf_pool = ctx.enter_context(tc.tile_pool(name="wbf_pool", bufs=2))
        wf32_pool = ctx.enter_context(tc.tile_pool(name="wf32_pool", bufs=2))
        wt_pool = ctx.enter_context(tc.tile_pool(name="wt_pool", bufs=2))
        cs_pool = ctx.enter_context(tc.tile_pool(name="cs_pool", bufs=2))
        small = ctx.enter_context(tc.tile_pool(name="small", bufs=4))

        encT_sb = []
        decT_sb = []
        enc_sb = []
        for k in range(kh):
            t_ = resident.tile([P, s_dim], BF16, name=f"encT_sb{k}", tag=f"encT{k}")
            nc.sync.dma_start(t_[:], encT_d[k * P : (k + 1) * P, :])
            encT_sb.append(t_)
        for k in range(kh):
            t_ = resident.tile([P, t_dim], BF16, name=f"decT_sb{k}", tag=f"decT{k}")
            nc.sync.dma_start(t_[:], decT_d[k * P : (k + 1) * P, :])
            decT_sb.append(t_)
        for j in range(ks):
            t_ = resident.tile([P, h_dim], BF16, name=f"enc_sb{j}", tag=f"enc{j}")
            nc.sync.dma_start(t_[:], enc_d[j * P : (j + 1) * P, :])
            enc_sb.append(t_)

        state = {}

        def mm1(i):
            t0 = i * P
            eb = [
                epool.tile([P, NBANK], F32, name=f"eb{i}_{n}", tag=f"e{n}")
                for n in range(ns)
            ]
            for k in range(kh):
                lhsT = decT_sb[k][:, t0 : t0 + P]
                for n in range(ns):
                    nc.tensor.matmul(
                        eb[n][:],
                        lhsT,
                        encT_sb[k][:, n * NBANK : (n + 1) * NBANK],
                        start=(k == 0),
                        stop=(k == kh - 1),
                    )
            state[i] = {"eb": eb}

        def softmax(i):
            st = state[i]
            eb = st["eb"]
            maxs = small.tile([P, ns], F32, name=f"maxs{i}", tag="maxs")
            for n in range(ns):
                nc.vector.tensor_reduce(maxs[:, n : n + 1], eb[n][:], AX_X, OP_MAX)
            negmax = small.tile([P, 1], F32, name=f"negmax{i}", tag="negmax")
            nc.vector.tensor_reduce(negmax[:], maxs[:], AX_X, OP_MAX, negate=True)

            wbf = wbf_pool.tile([P, s_dim], BF16, name=f"wbf{i}", tag="wbf")
            sums = small.tile([P, ns], F32, name=f"sums{i}", tag="sums")
            for n in range(ns):
                nc.scalar.activation(
                    wbf[:, n * NBANK : (n + 1) * NBANK],
                    eb[n][:],
                    ACT_EXP,
                    bias=negmax[:, 0:1],
                    accum_out=sums[:, n : n + 1],
                )
            ssum = small.tile([P, 1], F32, name=f"ssum{i}", tag="ssum")
            nc.vector.tensor_reduce(ssum[:], sums[:], AX_X, OP_ADD)
            r = small.tile([P, 1], F32, name=f"r{i}", tag="r")
            nc.vector.reciprocal(r[:], ssum[:])

            # normalized fp32 attention weights -> DRAM
            wf = wf32_pool.tile([P, s_dim], F32, name=f"wf{i}", tag="wf")
            nc.scalar.activation(wf[:], wbf[:], ACT_COPY, scale=r[:, 0:1])
            nc.sync.dma_start(wout_d[i * P : (i + 1) * P, :], wf[:])

            # transposed bf16 weights for MM2 (DMA xbar transpose, SBUF->SBUF)
            wt = wt_pool.tile([P, ks, P], BF16, name=f"wt{i}", tag="wt")
            for j in range(ks):
                nc.sync.dma_start(
                    wt[:, j, :], wbf[:, j * P : (j + 1) * P], transpose=True
                )
            st["wt"] = wt
            st["r"] = r

        def mm2(i):
            st = state.pop(i)
            wt = st["wt"]
            cb = cpool.tile([P, h_dim], F32, name=f"cb{i}", tag="cb")
            for j in range(ks):
                lhsT = wt[:, j, :]
                for n in range(nh):
                    nc.tensor.matmul(
                        cb[:, n * NBANK : (n + 1) * NBANK],
                        lhsT,
                        enc_sb[j][:, n * NBANK : (n + 1) * NBANK],
                        start=(j == 0),
                        stop=(j == ks - 1),
                    )
            cs = cs_pool.tile([P, h_dim], F32, name=f"cs{i}", tag="cs")
            nc.scalar.activation(cs[:], cb[:], ACT_COPY, scale=st["r"][:, 0:1])
            nc.sync.dma_start(cout_d[i * P : (i + 1) * P, :], cs[:])

        # software pipeline: PE stream = MM1(0) MM1(1) [MM1(i+1) MM2(i-1)]...
        mm1(0)
        softmax(0)
        if nt > 1:
            mm1(1)
        for i in range(1, nt):
            softmax(i)
            if i + 1 < nt:
                mm1(i + 1)
            mm2(i - 1)
        mm2(nt - 1)

    nc.compile()
    return nc


_NC_CACHE = {}


def _get_nc(shape_key):
    if shape_key not in _NC_CACHE:
        _NC_CACHE[shape_key] = build_kernel(*shape_key)
    return _NC_CACHE[shape_key]


def kernel(encoder_output, decoder_hidden, _trace=False, _tmpdir=None):
    encoder_output = np.asarray(encoder_output)
    decoder_hidden = np.asarray(decoder_hidden)
    s_dim, b_dim, h_dim = encoder_output.shape
    t_dim = decoder_hidden.shape[0]

    nc = _get_nc((t_dim, s_dim, h_dim))

    bf = ml_dtypes.bfloat16
    in_maps = []
    for b in range(b_dim):
        enc_b = np.ascontiguousarray(encoder_output[:, b, :]).astype(bf)
        dec_b = np.ascontiguousarray(decoder_hidden[:, b, :]).astype(bf)
        in_maps.append(
            {
                "encT": np.ascontiguousarray(enc_b.T),
                "decT": np.ascontiguousarray(dec_b.T),
                "enc": enc_b,
            }
        )

    res = run_bass_kernel_spmd(
        nc, in_maps, core_ids=list(range(b_dim)), trace=_trace, tmpdir=_tmpdir
    )
    kernel.last_results = res

    context = np.empty((t_dim, b_dim, h_dim), dtype=np.float32)
    weights = np.empty((t_dim, b_dim, s_dim), dtype=np.float32)
    for b in range(b_dim):
        context[:, b, :] = res.results[b]["context"]
        weights[:, b, :] = res.results[b]["weights"]

    kernel.last_exec_time_ns = res.exec_time_ns
    return (context, weights)


# revision 3
# speedup vs baseline: 1.8783x; 1.8783x over previous
"""Bass/Tile kernel for batched cross-attention on 8 TRN2 NeuronCores.

Problem (per reference):
    encoder_output: [S=2048, B=8, H=1024] f32
    decoder_hidden: [T=2048, B=8, H=1024] f32
    energies[b,t,s] = dec[t,b,:] . enc[s,b,:]
    weights = softmax(energies, axis=s)           -> returned as [T, B, S]
    context[t,b,:] = sum_s weights[b,t,s]*enc[s,b,:]  -> [T, B, H]

Sharding: batch b -> core b (pure data parallel, no collectives).

Per-core pipeline (T tiles of 128 rows):
    MM1 (TensorE, bf16):  E = dec_b @ enc_b^T     [128, S] fp32 in PSUM
    VectorE:              rowmax -> negmax
    ScalarE:              w = exp(E - max) (bf16) + accumulated rowsum
    DMA xbar:             w^T tiles for MM2
    TensorE (bf16):       C = w @ enc_b           [128, H] fp32 in PSUM
    ScalarE:              weights_out = w * (1/sum)  (f32), ctx_out = C * (1/sum)
The t-tile loop is software-pipelined (MM1 of tile i+1 is issued before
MM2 of tile i-1) so the PE stream never waits on the softmax/transpose.
"""

import os
import sys

for _p in ("/opt/trn_rl_repo", "/root/.axon_site/_ro/trn_rl_repo"):
    if os.path.isdir(_p) and _p not in sys.path:
        sys.path.insert(0, _p)

from contextlib import ExitStack

import ml_dtypes
import numpy as np

import concourse.bass as bass
import concourse.tile as tile
from concourse import bacc, mybir
from concourse.bass_utils import run_bass_kernel_spmd

S, T, B, H = 2048, 2048, 8, 1024
P = 128  # SBUF partitions
NBANK = 512  # fp32 PSUM bank free size / max moving free dim

BF16 = mybir.dt.bfloat16
F32 = mybir.dt.float32
AX_X = mybir.AxisListType.X
OP_MAX = mybir.AluOpType.max
OP_ADD = mybir.AluOpType.add
ACT_EXP = mybir.ActivationFunctionType.Exp
ACT_COPY = mybir.ActivationFunctionType.Copy


def build_kernel(t_dim=T, s_dim=S, h_dim=H):
    """Build and compile the per-core Bass module (same program on all cores)."""
    assert t_dim % P == 0 and s_dim % NBANK == 0 and h_dim % NBANK == 0

    kh = h_dim // P  # contraction chunks for MM1
    ks = s_dim // P  # contraction chunks for MM2
    nt = t_dim // P  # t tiles
    ns = s_dim // NBANK  # PSUM banks for energies
    nh = h_dim // NBANK  # PSUM-bank halves for context

    nc = bacc.Bacc("TRN2", target_bir_lowering=False, debug=False, num_devices=8)

    encT_d = nc.dram_tensor("encT", [h_dim, s_dim], BF16, kind="ExternalInput").ap()
    decT_d = nc.dram_tensor("decT", [h_dim, t_dim], BF16, kind="ExternalInput").ap()
    enc_d = nc.dram_tensor("enc", [s_dim, h_dim], BF16, kind="ExternalInput").ap()
    wout_d = nc.dram_tensor("weights", [t_dim, s_dim], F32, kind="ExternalOutput").ap()
    cout_d = nc.dram_tensor("context", [t_dim, h_dim], F32, kind="ExternalOutput").ap()

    with tile.TileContext(nc) as tc, ExitStack() as ctx:
        resident = ctx.enter_context(tc.tile_pool(name="resident", bufs=1))
        epool = ctx.enter_context(tc.tile_pool(name="epool", bufs=1, space="PSUM"))
        cpool = ctx.enter_context(tc.tile_pool(name="cpool", bufs=1, space="PSUM"))
        wbf_pool = ctx.enter_context(tc.tile_pool(name="wbf_pool", bufs=2))
        wf32_pool = ctx.enter_context(tc.tile_pool(name="wf32_pool", bufs=2))
        wt_pool = ctx.enter_context(tc.tile_pool(name="wt_pool", bufs=2))
        cs_pool = ctx.enter_context(tc.tile_pool(name="cs_pool", bufs=2))
        small = ctx.enter_context(tc.tile_pool(name="small", bufs=4))

        encT_sb = []
        decT_sb = []
        enc_sb = []
        for k in range(kh):
            t_ = resident.tile([P, s_dim], BF16, name=f"encT_sb{k}", tag=f"encT{k}")
            nc.sync.dma_start(t_[:], encT_d[k * P : (k + 1) * P, :])
            encT_sb.append(t_)
        for k in range(kh):
            t_ = resident.tile([P, t_dim], BF16, name=f"decT_sb{k}", tag=f"decT{k}")
            nc.sync.dma_start(t_[:], decT_d[k * P : (k + 1) * P, :])
            decT_sb.append(t_)
        for j in range(ks):
            t_ = resident.tile([P, h_dim], BF16, name=f"enc_sb{j}", tag=f"enc{j}")
            nc.sync.dma_start(t_[:], enc_d[j * P : (j + 1) * P, :])
            enc_sb.append(t_)

        state = {}

        def mm1(i):
            t0 = i * P
            eb = [
                epool.tile([P, NBANK], F32, name=f"eb{i}_{n}", tag=f"e{n}")
                for n in range(ns)
            ]
            for k in range(kh):
                lhsT = decT_sb[k][:, t0 : t0 + P]
                for n in range(ns):
                    nc.tensor.matmul(
                        eb[n][:],
                        lhsT,
                        encT_sb[k][:, n * NBANK : (n + 1) * NBANK],
                        start=(k == 0),
                        stop=(k == kh - 1),
                    )
            state[i] = {"eb": eb}

        def softmax(i):
            st = state[i]
            eb = st["eb"]
            maxs = small.tile([P, ns], F32, name=f"maxs{i}", tag="maxs")
            for n in range(ns):
                nc.vector.tensor_reduce(maxs[:, n : n + 1], eb[n][:], AX_X, OP_MAX)
            negmax = small.tile([P, 1], F32, name=f"negmax{i}", tag="negmax")
            nc.vector.tensor_reduce(negmax[:], maxs[:], AX_X, OP_MAX, negate=True)

            wbf = wbf_pool.tile([P, s_dim], BF16, name=f"wbf{i}", tag="wbf")
            sums = small.tile([P, ns], F32, name=f"sums{i}", tag="sums")
            for n in range(ns):
                nc.scalar.activation(
                    wbf[:, n * NBANK : (n + 1) * NBANK],
                    eb[n][:],
                    ACT_EXP,
                    bias=negmax[:, 0:1],
                    accum_out=sums[:, n : n + 1],
                )
            ssum = small.tile([P, 1], F32, name=f"ssum{i}", tag="ssum")
            nc.vector.tensor_reduce(ssum[:], sums[:], AX_X, OP_ADD)
            r = small.tile([P, 1], F32, name=f"r{i}", tag="r")
            nc.vector.reciprocal(r[:], ssum[:])

            # normalized fp32 attention weights -> DRAM
            wf = wf32_pool.tile([P, s_dim], F32, name=f"wf{i}", tag="wf")
            nc.scalar.activation(wf[:], wbf[:], ACT_COPY, scale=r[:, 0:1])
            nc.sync.dma_start(wout_d[i * P : (i + 1) * P, :], wf[:])

            # transposed bf16 weights for MM2 (DMA xbar transpose, SBUF->SBUF).
            # One call: out[:, j, :] == transpose(wbf[:, j*P:(j+1)*P]) (verified).
            wt = wt_pool.tile([P, ks, P], BF16, name=f"wt{i}", tag="wt")
            nc.sync.dma_start(wt[:], wbf[:], transpose=True)
            st["wt"] = wt
            st["r"] = r

        def mm2(i):
            st = state.pop(i)
            wt = st["wt"]
            cb = cpool.tile([P, h_dim], F32, name=f"cb{i}", tag="cb")
            for j in range(ks):
                lhsT = wt[:, j, :]
                for n in range(nh):
                    nc.tensor.matmul(
                        cb[:, n * NBANK : (n + 1) * NBANK],
                        lhsT,
                        enc_sb[j][:, n * NBANK : (n + 1) * NBANK],
                        start=(j == 0),
                        stop=(j == ks - 1),
                    )
            cs = cs_pool.tile([P, h_dim], F32, name=f"cs{i}", tag="cs")
            nc.scalar.activation(cs[:], cb[:], ACT_COPY, scale=st["r"][:, 0:1])
            nc.sync.dma_start(cout_d[i * P : (i + 1) * P, :], cs[:])

        # software pipeline: PE stream = MM1(0) MM1(1) [MM1(i+1) MM2(i-1)]...
        mm1(0)
        softmax(0)
        if nt > 1:
            mm1(1)
        for i in range(1, nt):
            softmax(i)
            if i + 1 < nt:
                mm1(i + 1)
            mm2(i - 1)
        mm2(nt - 1)

    nc.compile()
    return nc


_NC_CACHE = {}


def _get_nc(shape_key):
    if shape_key not in _NC_CACHE:
        _NC_CACHE[shape_key] = build_kernel(*shape_key)
    return _NC_CACHE[shape_key]


def kernel(encoder_output, decoder_hidden, _trace=False, _tmpdir=None):
    encoder_output = np.asarray(encoder_output)
    decoder_hidden = np.asarray(decoder_hidden)
    s_dim, b_dim, h_dim = encoder_output.shape
    t_dim = decoder_hidden.shape[0]

    nc = _get_nc((t_dim, s_dim, h_dim))

    bf = ml_dtypes.bfloat16
    in_maps = []
    for b in range(b_dim):
        enc_b = np.ascontiguousarray(encoder_output[:, b, :]).astype(bf)
        dec_b = np.ascontiguousarray(decoder_hidden[:, b, :]).astype(bf)
        in_maps.append(
            {
                "encT": np.ascontiguousarray(enc_b.T),
                "decT": np.ascontiguousarray(dec_b.T),
                "enc": enc_b,
            }
        )

    res = run_bass_kernel_spmd(
        nc, in_maps, core_ids=list(range(b_dim)), trace=_trace, tmpdir=_tmpdir
    )
    kernel.last_results = res

    context = np.empty((t_dim, b_dim, h_dim), dtype=np.float32)
    weights = np.empty((t_dim, b_dim, s_dim), dtype=np.float32)
    for b in range(b_dim):
        context[:, b, :] = res.results[b]["context"]
        weights[:, b, :] = res.results[b]["weights"]

    kernel.last_exec_time_ns = res.exec_time_ns
    return (context, weights)
